# revision 6
# baseline (speedup 1.0000x reference)
"""nn_Cate3Classifier — 8-core Trainium2 Bass kernel.

Math (see reference):
    h   = swem_vec @ W_fc (+ b_fc)        # b_fc cancels inside BatchNorm
    hn  = relu((h - mu) * rsqrt(var + eps) * gamma + beta)   # batch stats over ALL B rows
    out = hn @ W_clf + b_clf
    out[i, j] = -100 where mask2[cate2[i], j]

Distribution: pure data-parallel over the batch (2048 rows/core on 8 cores).
Weights/masks replicated. BN batch statistics use one tiny AllReduce of
per-core [sum_h, sum_h^2] (f32 [128, 8] = 4 KiB).

Per-core layout: hT "feature-major" [128 feat x rows] so that
  - fc matmul:  lhsT = W_fc chunk (native layout), rhs = x^T (bf16 DMA-transpose)
  - BN stats:   free-dim reductions (DVE/ACT accum_out)
  - BN apply:   one ACT op per tile: relu(h*s + t) with per-partition s,t
  - clf matmul: lhsT = hn^T chunk (already in SBUF), rhs = W_clf (native)
    -> output lands in natural [rows x 125] layout, no output transpose.
Masking: indirect-DMA row gather of a f32 0/1 table by cate2, then
copy_predicated with a -100 constant tile.
"""

import numpy as np
import ml_dtypes

B, D, H, C3, C2 = 16384, 2048, 512, 125, 64
NCORES = 8
BL = B // NCORES      # 2048 rows per core
RK = D // 128         # 16 contraction chunks (fc)
RF = H // 128         # 4 feature chunks
NRC = BL // 512       # 4 row chunks of 512
NT = BL // 128        # 16 output row tiles of 128
BN_EPS = 1e-5
MASK_VAL = -100.0

_CACHE = {}


def _build_nc():
    from contextlib import ExitStack

    import concourse.bass as bass
    import concourse.tile as tile
    from concourse import bacc, mybir

    f32 = mybir.dt.float32
    bf16 = mybir.dt.bfloat16
    i32 = mybir.dt.int32
    AF = mybir.ActivationFunctionType
    OP = mybir.AluOpType

    nc = bacc.Bacc("TRN2", target_bir_lowering=False, debug=False, num_devices=NCORES)

    x_d = nc.dram_tensor("x", [BL, D], bf16, kind="ExternalInput")
    wfc_d = nc.dram_tensor("wfc", [D, H], bf16, kind="ExternalInput")
    wclf_d = nc.dram_tensor("wclf", [H, C3], bf16, kind="ExternalInput")
    bclf_d = nc.dram_tensor("bclf", [128, C3], f32, kind="ExternalInput")
    gam_d = nc.dram_tensor("gam", [128, RF], f32, kind="ExternalInput")
    bet_d = nc.dram_tensor("bet", [128, RF], f32, kind="ExternalInput")
    m2_d = nc.dram_tensor("m2", [C2, C3], mybir.dt.uint8, kind="ExternalInput")
    cat_d = nc.dram_tensor("cat", [128, NT], i32, kind="ExternalInput")
    out_d = nc.dram_tensor("out", [BL, C3], f32, kind="ExternalOutput")

    with tile.TileContext(nc) as tc, ExitStack() as ctx:
        xpool = ctx.enter_context(tc.tile_pool(name="xT", bufs=RK))
        wpool = ctx.enter_context(tc.tile_pool(name="w", bufs=1))
        hpool = ctx.enter_context(tc.tile_pool(name="h", bufs=RF))
        hnpool = ctx.enter_context(tc.tile_pool(name="hn", bufs=RF))
        hsqpool = ctx.enter_context(tc.tile_pool(name="hsq", bufs=2))
        smallpool = ctx.enter_context(tc.tile_pool(name="small", bufs=1))
        outpool = ctx.enter_context(tc.tile_pool(name="outsb", bufs=4))
        psum_fc = ctx.enter_context(tc.tile_pool(name="psfc", bufs=6, space="PSUM"))
        psum_clf = ctx.enter_context(tc.tile_pool(name="psclf", bufs=2, space="PSUM"))
        drampool = ctx.enter_context(tc.tile_pool(name="dram", bufs=1, space="DRAM"))

        # ---- constant / parameter loads ----
        wfc_sb = wpool.tile([128, RK * H], bf16, tag="wfc")
        nc.sync.dma_start(
            wfc_sb[:].rearrange("p (k h) -> p k h", k=RK),
            wfc_d.ap().rearrange("(k p) h -> p k h", p=128),
        )
        wclf_sb = wpool.tile([128, RF * C3], bf16, tag="wclf")
        nc.sync.dma_start(
            wclf_sb[:].rearrange("p (f c) -> p f c", f=RF),
            wclf_d.ap().rearrange("(f p) c -> p f c", p=128),
        )
        bclf_sb = smallpool.tile([128, C3], f32, tag="bclf")
        nc.sync.dma_start(bclf_sb[:], bclf_d.ap())
        gam_sb = smallpool.tile([128, RF], f32, tag="gam")
        nc.sync.dma_start(gam_sb[:], gam_d.ap())
        bet_sb = smallpool.tile([128, RF], f32, tag="bet")
        nc.sync.dma_start(bet_sb[:], bet_d.ap())
        cate_sb = smallpool.tile([128, NT], i32, tag="cate")
        nc.sync.dma_start(cate_sb[:], cat_d.ap())

        neg100 = smallpool.tile([128, C3], f32, tag="neg100")
        nc.vector.memset(neg100[:], MASK_VAL)

        # ---- mask row gather (independent of the matmul chain) ----
        mask_sb = smallpool.tile([128, NT * C3], mybir.dt.uint8, tag="mask")
        for t in range(NT):
            nc.gpsimd.indirect_dma_start(
                out=mask_sb[:, t * C3 : (t + 1) * C3],
                out_offset=None,
                in_=m2_d.ap(),
                in_offset=bass.IndirectOffsetOnAxis(ap=cate_sb[:, t : t + 1], axis=0),
            )

        # ---- x^T tiles via HW DMA transpose (bf16) ----
        xts = []
        for k in range(RK):
            xk = xpool.tile([128, BL], bf16, tag="xT", name=f"xT{k}")
            nc.sync.dma_start(xk[:], x_d.ap()[:, k * 128 : (k + 1) * 128], transpose=True)
            xts.append(xk)

        # ---- fc matmul + streaming BN stats ----
        h_sb = [hpool.tile([128, BL], bf16, tag="h", name=f"h{f}") for f in range(RF)]
        sums_sb = smallpool.tile([128, RF * NRC], f32, tag="sums")
        sumsq_sb = smallpool.tile([128, RF * NRC], f32, tag="sumsq")

        for f in range(RF):
            psums = [psum_fc.tile([128, 512], f32, tag="ps", name=f"psfc{f}_{r}") for r in range(NRC)]
            for k in range(RK):
                lhsT = wfc_sb[:, k * H + f * 128 : k * H + (f + 1) * 128]
                for r in range(NRC):
                    nc.tensor.matmul(
                        psums[r][:],
                        lhsT=lhsT,
                        rhs=xts[k][:, r * 512 : (r + 1) * 512],
                        start=(k == 0),
                        stop=(k == RK - 1),
                    )
            for r in range(NRC):
                col = f * NRC + r
                # copy h psum -> SBUF bf16, accumulating per-feature sum
                nc.vector.tensor_scalar(
                    out=h_sb[f][:, r * 512 : (r + 1) * 512],
                    in0=psums[r][:],
                    scalar1=1.0,
                    scalar2=None,
                    op0=OP.mult,
                    op1=OP.add,
                    accum_out=sums_sb[:, col : col + 1],
                )
                # h^2 (discarded) + per-feature sum of squares on ACT
                hsq = hsqpool.tile([128, 512], bf16, tag="hsq")
                nc.scalar.activation(
                    out=hsq[:],
                    in_=psums[r][:],
                    func=AF.Square,
                    accum_out=sumsq_sb[:, col : col + 1],
                )

        # ---- combine per-core stats, AllReduce across cores ----
        stats_sb = smallpool.tile([128, 2 * RF], f32, tag="stats")
        for f in range(RF):
            nc.vector.reduce_sum(
                out=stats_sb[:, f : f + 1],
                in_=sums_sb[:, f * NRC : (f + 1) * NRC],
                axis=mybir.AxisListType.X,
            )
            nc.vector.reduce_sum(
                out=stats_sb[:, RF + f : RF + f + 1],
                in_=sumsq_sb[:, f * NRC : (f + 1) * NRC],
                axis=mybir.AxisListType.X,
            )
        cc_in = drampool.tile([128, 2 * RF], f32, tag="ccin")
        cc_out = drampool.tile([128, 2 * RF], f32, tag="ccout")
        nc.sync.dma_start(cc_in[:], stats_sb[:])
        nc.gpsimd.collective_compute(
            "AllReduce",
            OP.add,
            replica_groups=[list(range(NCORES))],
            ins=[cc_in[:].opt()],
            outs=[cc_out[:].opt()],
        )
        stats_all = smallpool.tile([128, 2 * RF], f32, tag="statsall")
        nc.sync.dma_start(stats_all[:], cc_out[:])

        # ---- BN scale/shift: s = gamma*rsqrt(var+eps), t = beta - mean*s ----
        mean = smallpool.tile([128, RF], f32, tag="mean")
        nc.vector.tensor_scalar_mul(mean[:], stats_all[:, 0:RF], 1.0 / B)
        ex2 = smallpool.tile([128, RF], f32, tag="ex2")
        nc.vector.tensor_scalar_mul(ex2[:], stats_all[:, RF : 2 * RF], 1.0 / B)
        msq = smallpool.tile([128, RF], f32, tag="msq")
        nc.vector.tensor_tensor(out=msq[:], in0=mean[:], in1=mean[:], op=OP.mult)
        var = smallpool.tile([128, RF], f32, tag="var")
        nc.vector.tensor_tensor(out=var[:], in0=ex2[:], in1=msq[:], op=OP.subtract)
        eps_sb = smallpool.tile([128, 1], f32, tag="eps")
        nc.vector.memset(eps_sb[:], BN_EPS)
        std = smallpool.tile([128, RF], f32, tag="std")
        nc.scalar.activation(std[:], var[:], AF.Sqrt, bias=eps_sb[:, 0:1])
        rstd = smallpool.tile([128, RF], f32, tag="rstd")
        nc.vector.reciprocal(rstd[:], std[:])
        svec = smallpool.tile([128, RF], f32, tag="svec")
        nc.vector.tensor_tensor(out=svec[:], in0=gam_sb[:], in1=rstd[:], op=OP.mult)
        mstmp = smallpool.tile([128, RF], f32, tag="mstmp")
        nc.vector.tensor_tensor(out=mstmp[:], in0=mean[:], in1=svec[:], op=OP.mult)
        tvec = smallpool.tile([128, RF], f32, tag="tvec")
        nc.vector.tensor_tensor(out=tvec[:], in0=bet_sb[:], in1=mstmp[:], op=OP.subtract)

        # ---- BN apply + relu, then clf matmul + bias + mask + store ----
        hn_sb = [hnpool.tile([128, BL], bf16, tag="hn", name=f"hn{f}") for f in range(RF)]
        for r in range(NRC):
            for f in range(RF):
                nc.scalar.activation(
                    out=hn_sb[f][:, r * 512 : (r + 1) * 512],
                    in_=h_sb[f][:, r * 512 : (r + 1) * 512],
                    func=AF.Relu,
                    scale=svec[:, f : f + 1],
                    bias=tvec[:, f : f + 1],
                )
            for sub in range(4):
                t = r * 4 + sub
                po = psum_clf.tile([128, C3], f32)
                for f in range(RF):
                    nc.tensor.matmul(
                        po[:],
                        lhsT=hn_sb[f][:, t * 128 : (t + 1) * 128],
                        rhs=wclf_sb[:, f * C3 : (f + 1) * C3],
                        start=(f == 0),
                        stop=(f == RF - 1),
                    )
                out_t = outpool.tile([128, C3], f32, tag="outt")
                nc.vector.tensor_tensor(out=out_t[:], in0=po[:], in1=bclf_sb[:], op=OP.add)
                nc.vector.copy_predicated(out_t[:], mask_sb[:, t * C3 : (t + 1) * C3], neg100[:])
                nc.sync.dma_start(out_d.ap()[t * 128 : (t + 1) * 128, :], out_t[:])

    nc.compile()
    return nc


def _get_nc():
    if "nc" not in _CACHE:
        _CACHE["nc"] = _build_nc()
    return _CACHE["nc"]


def make_in_maps(**inputs):
    """Host-side marshaling: shard/cast the full inputs into per-core maps."""
    bf16 = ml_dtypes.bfloat16
    x = np.ascontiguousarray(np.asarray(inputs["swem_vec"], dtype=np.float32)).astype(bf16)
    wfc = np.asarray(inputs["W_fc"], dtype=np.float32).astype(bf16)
    wclf = np.asarray(inputs["W_clf"], dtype=np.float32).astype(bf16)
    bclf = np.tile(np.asarray(inputs["b_clf"], dtype=np.float32)[None, :], (128, 1))
    gam = np.ascontiguousarray(
        np.asarray(inputs["gamma"], dtype=np.float32).reshape(RF, 128).T
    )
    bet = np.ascontiguousarray(
        np.asarray(inputs["beta"], dtype=np.float32).reshape(RF, 128).T
    )
    m2 = np.asarray(inputs["mask2"]).astype(np.uint8)
    cate = np.asarray(inputs["cate2"]).astype(np.int32)

    in_maps = []
    for c in range(NCORES):
        sl = slice(c * BL, (c + 1) * BL)
        in_maps.append(
            {
                "x": np.ascontiguousarray(x[sl]),
                "wfc": wfc,
                "wclf": wclf,
                "bclf": bclf,
                "gam": gam,
                "bet": bet,
                "m2": m2,
                "cat": np.ascontiguousarray(cate[sl].reshape(NT, 128).T),
            }
        )
    return in_maps


def run(in_maps, trace=False, **kwargs):
    from concourse.bass_utils import run_bass_kernel_spmd

    nc = _get_nc()
    return run_bass_kernel_spmd(
        nc, in_maps, core_ids=list(range(NCORES)), trace=trace, **kwargs
    )


def kernel(**inputs) -> np.ndarray:
    in_maps = make_in_maps(**inputs)
    res = run(in_maps, trace=False)
    return np.concatenate([res.results[c]["out"] for c in range(NCORES)], axis=0)
